# revision 1
# baseline (speedup 1.0000x reference)
"""HeteroMoE layer for Trainium2, 8-core SPMD — multi-engine fp16 edition.

Top-1 routing with weight exactly 1.0, so out[b] = expert_{argmax(logits[b])}(x[b]).
Host routes, permutes the batch into 8 cores x 4 slots (uniform compile-time
mode per slot), BN-folds weights, pads + fp16-quantizes x on the host so the
device DMA lands directly in padded SBUF tiles (no memsets / copies).

The depthwise 3x3 taps dominate; work is split per 16-image-row "unit"
(slot, block, j) across engines, each engine owning an independent
accumulator so no cross-engine serial chains form:
 - PE units (most has_m units): diagonal fp16 matmuls into PSUM, with
   expert-2's 1x1 pre-matmul (center tap folded into its diagonal) in the
   same PSUM accumulation group.
 - DVE units: zacc chain of tensor_scalar products (4x packed) +
   tensor_tensor adds (2x packed); npo taps' DVE products are
   accumulated into zpo by GPSIMD tensor_tensor adds; the scalar engine
   produces nsc products consumed by DVE adds. The zacc += zpo merge, optional PSUM
   inject (identity matmul, has_m only), and the unit's gelu are deferred
   several units for pipelining.
Slots are processed round-robin by j so PE-heavy (has_m) and DVE/Pool-heavy
(pure-d) units overlap. Gelu+BN-bias on the scalar engine, 1x1 pointwise as
PE matmuls (lagging stage 1), stage-2 bias readout split scalar/DVE, fp16
output DMA per (slot, j).
"""
import numpy as np

import concourse.bacc as bacc
import concourse.tile as tile
import concourse.mybir as mybir
from concourse.ap import AP
from concourse.bass_utils import run_bass_kernel_spmd

F32 = mybir.dt.float32
F16 = mybir.dt.float16

B, C, H, W = 32, 256, 64, 64
HW = H * W
NCORES = 8
NSLOT = B // NCORES
PAD = 2
R = W + 2 * PAD          # 68
RH = H + 2 * PAD         # 68
PADHW = R * RH           # 4624
NBLK = 2
EPS = 1e-5
NJ = 4                   # units per (slot, block): 16 image rows each

_CACHE = {}
LAST_NC = None

# n_hm_dve: has_m units moved off PE to DVE (PSUM inject path)
# n_pd_pe:  pure-d units moved onto PE
# npo: taps per DVE unit accumulated on gpsimd (fused STT chain)
# nsc: products per DVE unit done on the scalar engine
# rod: fraction of stage-2 readouts on DVE
# lagf/lags: finish / stage-2 queue depths (units / (s,j) entries)
KNOBS = dict(n_hm_dve=2, n_pd_pe=2, npo=3, npp=0, npof=0.55, nsp=0.0,
             nsc=2, rod=0.1, rot=10, lagf=3, lags=2, pro=2)


def _offsets(tm, has_m):
    """[(dy,dx,dil)] tap list; center dropped when has_m (folded into M)."""
    offs = []
    dils = {"d1": [1], "d2": [2], "d12": [1, 2], None: []}[tm]
    for i, dil in enumerate(dils):
        for dy in (-1, 0, 1):
            for dx in (-1, 0, 1):
                if (dy, dx) == (0, 0) and (has_m or i > 0):
                    continue
                offs.append((dy, dx, dil))
    return offs


class _Frac:
    def __init__(self, frac):
        self.frac = frac
        self.n = 0
        self.hit = 0

    def __call__(self):
        self.n += 1
        if self.hit < int(self.frac * self.n + 0.5):
            self.hit += 1
            return True
        return False


def build(slot_modes, knobs=None):
    kn = dict(KNOBS)
    if knobs:
        kn.update(knobs)
    nc = bacc.Bacc("TRN2", target_bir_lowering=False, debug=False,
                   num_devices=NCORES)
    xin = nc.dram_tensor("xin", [NSLOT, C, PADHW], F16,
                         kind="ExternalInput").ap()
    yout = nc.dram_tensor("yout", [NSLOT, C, HW], F16,
                          kind="ExternalOutput").ap()
    prm = {"ident": nc.dram_tensor("ident", [128, 128], F16,
                                   kind="ExternalInput").ap()}
    live = [s for s, (tm, hm) in enumerate(slot_modes)
            if tm is not None or hm]
    for s in live:
        tm, has_m = slot_modes[s]
        ntap = len(_offsets(tm, has_m))
        if ntap:
            prm[f"dkd_{s}"] = nc.dram_tensor(
                f"dkd_{s}", [128, NBLK * ntap * 128], F16,
                kind="ExternalInput").ap()
            prm[f"dkv_{s}"] = nc.dram_tensor(
                f"dkv_{s}", [128, NBLK * ntap], F32,
                kind="ExternalInput").ap()
        if has_m:
            prm[f"pm_{s}"] = nc.dram_tensor(
                f"pm_{s}", [128, NBLK * NBLK * 128], F16,
                kind="ExternalInput").ap()
        prm[f"pw_{s}"] = nc.dram_tensor(
            f"pw_{s}", [128, NBLK * NBLK * 128], F16,
            kind="ExternalInput").ap()
        prm[f"pf_{s}"] = nc.dram_tensor(
            f"pf_{s}", [128, 4], F32, kind="ExternalInput").ap()

    hm_units = [(s, b, j) for j in range(NJ) for s in live
                if slot_modes[s][1] for b in range(NBLK)]
    pd_units = [(s, b, j) for j in range(NJ) for s in live
                if slot_modes[s][0] is not None and not slot_modes[s][1]
                for b in range(NBLK)]
    ho = kn.get("hmoff", 0)
    hm_dve = hm_units[ho:ho + kn["n_hm_dve"]]
    pe_units = set([u for u in hm_units if u not in hm_dve]
                   + pd_units[::-1][::kn.get("pdstride", 1)]
                   [:kn["n_pd_pe"]])
    if kn.get("pin0") and pd_units:
        pe_units.add(pd_units[0])
    f_rod = _Frac(kn["rod"])
    f_npo = _Frac(kn.get("npof", 0.0))
    f_nsp = _Frac(kn.get("nsp", 0.0))
    f_mpo = _Frac(kn.get("mpo", 0.0))
    n_ro = [0]
    n_dveu = [0]
    total_ro = 2 * len(live) * NJ

    def v2d(ap1d, rows):
        return ap1d.rearrange("p (a b) -> p a b", a=rows, b=64)

    with tile.TileContext(nc) as tc:
        with tc.tile_pool(name="params", bufs=1) as ppool, \
             tc.tile_pool(name="x16", bufs=1) as xpool, \
             tc.tile_pool(name="a16", bufs=8) as apool, \
             tc.tile_pool(name="o16", bufs=4) as opool, \
             tc.tile_pool(name="zacc", bufs=6) as zapool, \
             tc.tile_pool(name="zpo", bufs=6) as zppool, \
             tc.tile_pool(name="tmp", bufs=10) as tpool, \
             tc.tile_pool(name="warm", bufs=1) as wpool, \
             tc.tile_pool(name="psz", bufs=2, space="PSUM") as pszp, \
             tc.tile_pool(name="psw", bufs=2, space="PSUM") as pswp:

            pt = {}
            prm_done = set()
            for name, ap in prm.items():
                pt[name] = ppool.tile(list(ap.shape), ap.dtype, tag=name,
                                      name=name)

            n_prm = [0]

            def need_prm(name):
                if name in prm_done or name not in prm:
                    return
                prm_done.add(name)
                n_prm[0] += 1
                pq = kn.get("pq", 0)
                if pq == 1:
                    eng = nc.gpsimd
                elif pq == 2:
                    eng = nc.sync if n_prm[0] <= kn.get("pqn", 4) \
                        else nc.scalar
                elif pq == 3:
                    eng = nc.gpsimd if n_prm[0] <= kn.get("pqn", 4) \
                        else nc.scalar
                else:
                    eng = nc.scalar
                eng.dma_start(pt[name][:], prm[name])

            # slot contexts; x DMAs are emitted lazily per (slot, block,
            # row-half): piece 0 = padded rows [0, 38), piece 1 = [32, 68)
            dma_done = set()

            def need_x(s, b, j):
                for jh in range(min((j + kn.get("xpre", 0)) // 2, 1) + 1):
                    if (s, b, jh) in dma_done:
                        continue
                    dma_done.add((s, b, jh))
                    r0, r1 = (0, 38) if jh == 0 else (32, RH)
                    nc.sync.dma_start(
                        sctx[s]["x16"][:, b, r0 * R:r1 * R],
                        xin[s, b * 128:(b + 1) * 128, r0 * R:r1 * R])

            # Activation-table prewarm: touch every act func at t0 so
            # LoadActFuncSet instructions land while Act is idle
            if kn.get("actw", 0) == 2:
                wact = wpool.tile([128, 512], F16, tag="w", name="w")
                nc.gpsimd.memset(wact[:, 0:8], 0)
                nc.scalar.activation(wact[:, 0:4], wact[:, 4:8],
                                     mybir.ActivationFunctionType.Gelu,
                                     bias=0.0, scale=1.0)
            elif kn.get("actw", 0):
                wact = tpool.tile([128, 1024], F16, tag="t", name="t")
                nc.gpsimd.memset(wact[:, 0:8], 0)
                nc.scalar.activation(wact[:, 0:4], wact[:, 4:8],
                                     mybir.ActivationFunctionType.Gelu,
                                     bias=0.0, scale=1.0)
                nc.scalar.activation(wact[:, 0:4], wact[:, 4:8],
                                     mybir.ActivationFunctionType.Identity,
                                     bias=0.0, scale=1.0)
                nc.scalar.activation(wact[:, 0:4], wact[:, 4:8],
                                     mybir.ActivationFunctionType.Copy,
                                     bias=0.0, scale=1.0)

            # PE p-state warmup: dummy matmuls while input DMAs land
            if kn.get("warm", 0):
                wsrc = wpool.tile([128, 512], F16, tag="w", name="w")
                nc.gpsimd.memset(wsrc[:], 0)
                wp = pswp.tile([128, 1024], F32, tag="psw", name="psw")
                for _ in range(kn["warm"]):
                    nc.tensor.matmul(wp[:, 0:512], wsrc[:, 0:128],
                                     wsrc[:], start=True, stop=True)
                nc.vector.tensor_copy(wsrc[:].bitcast(F32),
                                      wp[:, 0:256])

            sctx = {}
            for s in live:
                tm, has_m = slot_modes[s]
                offs = _offsets(tm, has_m)
                ntap = len(offs)
                x16 = xpool.tile([128, NBLK, PADHW], F16, tag=f"x{s}",
                                 name=f"x{s}")
                d = dict(offs=offs, ntap=ntap, x16=x16, has_m=has_m,
                         pf=pt[f"pf_{s}"])
                if ntap:
                    d["dkd"] = pt[f"dkd_{s}"][:].rearrange(
                        "p (b t m) -> p b t m", b=NBLK, t=ntap)
                    d["dkv"] = pt[f"dkv_{s}"]
                if has_m:
                    d["pm"] = pt[f"pm_{s}"][:].rearrange(
                        "p (cb ib m) -> p cb ib m", cb=2, ib=2)
                d["pw"] = pt[f"pw_{s}"][:].rearrange(
                    "p (cb ib m) -> p cb ib m", cb=2, ib=2)
                sctx[s] = d

            def win(s, b, j, dy, dx, dil, nrows, r0=0):
                xap = sctx[s]["x16"][:]
                off = (b * PADHW + (PAD + 16 * j + 8 * r0 + dy * dil) * R
                       + PAD + dx * dil)
                return AP(xap.tensor, xap.offset + off,
                          [[xap.ap[0][0], 128], [R, nrows * 8], [1, 64]])

            a16j = {}   # (s, j) -> tile [128, NBLK, 1024]
            finish_q = []
            stage2_q = []

            def do_stage2(s, j):
                while any(e[0] == s and e[1] == j for e in finish_q):
                    finish_q.pop(0)[2]()
                d = sctx[s]
                need_prm(f"pw_{s}")
                a16 = a16j.pop((s, j))
                o16 = opool.tile([128, NBLK, 1024], F16, tag="o16",
                                 name="o16")
                for cb in range(NBLK):
                    psw = pswp.tile([128, 1024], F32, tag="psw", name="psw")
                    for half in range(2):
                        for ib in range(NBLK):
                            nc.tensor.matmul(
                                psw[:, half * 512:(half + 1) * 512],
                                d["pw"][:, cb, ib],
                                a16[:, ib, half * 512:(half + 1) * 512],
                                start=(ib == 0), stop=(ib == NBLK - 1))
                    dst = v2d(o16[:, cb, :], 16)
                    src = v2d(psw[:], 16)
                    n_ro[0] += 1
                    in_tail = n_ro[0] > total_ro - kn["rot"]
                    if (in_tail and n_ro[0] % 2 == 0) or \
                            (not in_tail and f_rod()):
                        nc.vector.tensor_scalar(
                            dst, src, d["pf"][:, 2 + cb:3 + cb], None,
                            op0=mybir.AluOpType.add)
                    else:
                        nc.scalar.activation(
                            dst, src, mybir.ActivationFunctionType.Identity,
                            bias=d["pf"][:, 2 + cb:3 + cb], scale=1.0)
                    if in_tail or kn.get("ocb", 0):
                        nc.sync.dma_start(
                            yout[s, cb * 128:(cb + 1) * 128,
                                 j * 1024:(j + 1) * 1024], o16[:, cb, :])
                if not (in_tail or kn.get("ocb", 0)):
                    nc.sync.dma_start(
                        yout[s, :, j * 1024:(j + 1) * 1024].rearrange(
                            "(b p) c -> p b c", b=NBLK), o16[:])

            def emit_unit(s, b, j):
                d = sctx[s]
                offs, ntap, has_m = d["offs"], d["ntap"], d["has_m"]
                need_x(s, b, j)
                if has_m:
                    need_x(s, 0, j)
                    need_x(s, 1, j)
                    need_prm(f"pm_{s}")
                need_prm(f"pf_{s}")
                if (s, b, j) in pe_units and ntap:
                    need_prm(f"dkd_{s}")
                elif ntap:
                    need_prm(f"dkv_{s}")
                    if has_m:
                        need_prm("ident")
                if (s, j) not in a16j:
                    a16j[(s, j)] = apool.tile([128, NBLK, 1024], F16,
                                              tag="a16", name="a16")
                adst = v2d(a16j[(s, j)][:, b, :], 16)
                gbias = d["pf"][:, b:b + 1]

                if ((s, b, j) in pe_units and ntap) or (ntap == 0 and has_m):
                    psz = pszp.tile([128, 1024], F32, tag="psz", name="psz")
                    for half in range(2):
                        out = psz[:, half * 512:(half + 1) * 512]
                        for t, (dy, dx, dil) in enumerate(offs):
                            nc.tensor.matmul(
                                out, d["dkd"][:, b, t],
                                win(s, b, j, dy, dx, dil, 1, half),
                                start=(t == 0),
                                stop=(not has_m and t == ntap - 1))
                        if has_m:
                            for ib in range(NBLK):
                                nc.tensor.matmul(
                                    out, d["pm"][:, b, ib],
                                    win(s, ib, j, 0, 0, 1, 1, half),
                                    start=(ntap == 0 and ib == 0),
                                    stop=(ib == NBLK - 1))
                    nc.scalar.activation(
                        adst, v2d(psz[:], 16),
                        mybir.ActivationFunctionType.Gelu,
                        bias=gbias, scale=1.0)
                    return

                # multi-engine DVE unit: npo taps -> DVE products + Pool
                # adds (zpo chain); npp products on Pool, nsc on scalar
                # (adds on DVE); rest fully on DVE (zacc chain).
                tl = list(enumerate(offs))
                n_dveu[0] += 1
                extra = (n_dveu[0] <= kn.get("npo4k", 0)) or f_npo()
                npo_u = kn["npo"] + (1 if extra else 0)
                nsc_u = kn["nsc"]
                if has_m and "nsch" in kn:
                    nsc_u = kn["nsch"]
                if has_m and "npoh" in kn:
                    npo_u = kn["npoh"]
                po_t = (tl[:npo_u]
                        if npo_u >= 2 and ntap > npo_u + 1 else [])
                rest = tl[len(po_t):]
                pp_t = rest[1:1 + kn["npp"]]
                sc_t = rest[1 + kn["npp"]:1 + kn["npp"] + kn["nsc"]]
                dv_t = [rest[0]] + rest[1 + kn["npp"] + kn["nsc"]:]
                zacc = zapool.tile([128, 1024], F16, tag="z", name="z")
                zpo = None
                po_tmp = []
                for t, (dy, dx, dil) in po_t:
                    sc = d["dkv"][:, b * ntap + t:b * ntap + t + 1]
                    tmp = tpool.tile([128, 1024], F16, tag="t", name="t")
                    nc.vector.tensor_scalar(
                        v2d(tmp[:], 16), win(s, b, j, dy, dx, dil, 2),
                        sc, None, op0=mybir.AluOpType.mult)
                    po_tmp.append(tmp)
                if po_t:
                    zpo = zppool.tile([128, 1024], F16, tag="zp", name="zp")
                    nc.gpsimd.tensor_tensor(
                        zpo[:], po_tmp[0][:], po_tmp[1][:],
                        op=mybir.AluOpType.add)
                    for tmp in po_tmp[2:]:
                        nc.gpsimd.tensor_tensor(
                            zpo[:], zpo[:], tmp[:], op=mybir.AluOpType.add)
                lat_tmp = []
                for t, (dy, dx, dil) in pp_t:
                    sc = d["dkv"][:, b * ntap + t:b * ntap + t + 1]
                    tmp = tpool.tile([128, 1024], F16, tag="t", name="t")
                    nc.gpsimd.tensor_scalar(
                        v2d(tmp[:], 16), win(s, b, j, dy, dx, dil, 2),
                        sc, None, op0=mybir.AluOpType.mult)
                    lat_tmp.append(tmp)
                for t, (dy, dx, dil) in sc_t:
                    sc = d["dkv"][:, b * ntap + t:b * ntap + t + 1]
                    tmp = tpool.tile([128, 1024], F16, tag="t", name="t")
                    nc.scalar.activation(
                        v2d(tmp[:], 16), win(s, b, j, dy, dx, dil, 2),
                        (mybir.ActivationFunctionType.Identity
                         if kn.get("pid") else
                         mybir.ActivationFunctionType.Copy),
                        bias=0.0, scale=sc)
                    if zpo is not None and f_nsp():
                        nc.gpsimd.tensor_tensor(
                            zpo[:], zpo[:], tmp[:], op=mybir.AluOpType.add)
                    else:
                        lat_tmp.append(tmp)
                first = True
                for t, (dy, dx, dil) in dv_t:
                    sc = d["dkv"][:, b * ntap + t:b * ntap + t + 1]
                    wap = win(s, b, j, dy, dx, dil, 2)
                    if first:
                        nc.vector.tensor_scalar(
                            v2d(zacc[:], 16), wap, sc, None,
                            op0=mybir.AluOpType.mult)
                        first = False
                        continue
                    tmp = tpool.tile([128, 1024], F16, tag="t", name="t")
                    nc.vector.tensor_scalar(
                        v2d(tmp[:], 16), wap, sc, None,
                        op0=mybir.AluOpType.mult)
                    nc.vector.tensor_tensor(
                        zacc[:], zacc[:], tmp[:], op=mybir.AluOpType.add)
                for tmp in lat_tmp:
                    nc.vector.tensor_tensor(
                        zacc[:], zacc[:], tmp[:], op=mybir.AluOpType.add)

                def finish(zacc=zacc, zpo=zpo, s=s, b=b, j=j, adst=adst,
                           gbias=gbias, d=d, has_m=has_m):
                    if zpo is not None:
                        meng = nc.gpsimd if f_mpo() else nc.vector
                        meng.tensor_tensor(
                            zacc[:], zacc[:], zpo[:], op=mybir.AluOpType.add)
                    if has_m:
                        psz = pszp.tile([128, 1024], F32, tag="psz",
                                        name="psz")
                        for half in range(2):
                            out = psz[:, half * 512:(half + 1) * 512]
                            for ib in range(NBLK):
                                nc.tensor.matmul(
                                    out, d["pm"][:, b, ib],
                                    win(s, ib, j, 0, 0, 1, 1, half),
                                    start=(ib == 0), stop=False)
                            nc.tensor.matmul(
                                out, pt["ident"][:],
                                zacc[:, half * 512:(half + 1) * 512],
                                start=False, stop=True)
                        nc.scalar.activation(
                            adst, v2d(psz[:], 16),
                            mybir.ActivationFunctionType.Gelu,
                            bias=gbias, scale=1.0)
                    else:
                        nc.scalar.activation(
                            adst, v2d(zacc[:], 16),
                            mybir.ActivationFunctionType.Gelu,
                            bias=gbias, scale=1.0)

                finish_q.append((s, j, finish))

            hm_slots = [s for s in live if slot_modes[s][1]]
            pd_slots = [s for s in live if not slot_modes[s][1]]
            order = []
            for i in range(max(len(hm_slots), len(pd_slots))):
                if i < len(hm_slots):
                    order.append(hm_slots[i])
                if i < len(pd_slots):
                    order.append(pd_slots[i])
            sj_list = [(s, j) for j in range(NJ) for s in order]
            if kn.get("pro") and hm_slots:
                # fill prologue: pull the first PE slot's j=1 group right
                # after its j=0 group (same x pieces, keeps PE fed while
                # the other slots' input DMAs stream in)
                s0_ = hm_slots[0]
                sj_list.remove((s0_, 1))
                sj_list.insert(sj_list.index((s0_, 0)) + 1 + kn["pro"] - 1,
                               (s0_, 1))
            for s, j in sj_list:
                lagf_now = (kn.get("lagft", kn["lagf"])
                            if j == NJ - 1 else kn["lagf"])
                for b in range(NBLK):
                    emit_unit(s, b, j)
                    while len(finish_q) > lagf_now:
                        finish_q.pop(0)[2]()
                stage2_q.append((s, j))
                lag_now = (kn.get("lagt", kn["lags"])
                           if j == NJ - 1 else kn["lags"])
                while len(stage2_q) > lag_now:
                    do_stage2(*stage2_q.pop(0))
            while finish_q:
                finish_q.pop(0)[2]()
            while stage2_q:
                do_stage2(*stage2_q.pop(0))
    nc.compile()
    return nc


def _plan(idx):
    """Assign elements to (core, slot); return slot_modes, elem[core][slot]."""
    by = [list(np.where(idx == t)[0]) for t in range(3)]
    groups = []
    for t, mode in ((0, "d1"), (1, "d2")):
        while len(by[t]) >= NCORES:
            groups.append([mode, False, by[t][:NCORES]])
            by[t] = by[t][NCORES:]
    for t, mode in ((0, "d1"), (1, "d2")):
        if by[t]:
            take = min(NCORES - len(by[t]), len(by[2]))
            g = by[t] + by[2][:take]
            by[2] = by[2][take:]
            by[t] = []
            groups.append([mode, take > 0, g])
    while by[2]:
        groups.append([None, True, by[2][:NCORES]])
        by[2] = by[2][NCORES:]
    while len(groups) > NSLOT:
        tapg = [g for g in groups if g[0] is not None]
        a, b = tapg[-2], tapg[-1]
        groups.remove(b)
        a[0] = "d12" if a[0] != b[0] else a[0]
        a[1] = a[1] or b[1]
        a[2] += b[2]
        assert len(a[2]) <= NCORES
    for g in groups:
        while len(g[2]) < NCORES:
            g[2].append(-1)
    while len(groups) < NSLOT:
        groups.append([None, False, [-1] * NCORES])
    slot_modes = tuple((g[0], g[1]) for g in groups)
    elem = [[groups[s][2][c] for s in range(NSLOT)] for c in range(NCORES)]
    return slot_modes, elem


def _fold_params(kw):
    out = {}
    for i in range(3):
        g = kw[f"e{i}_g"]; b = kw[f"e{i}_b"]
        m = kw[f"e{i}_m"]; v = kw[f"e{i}_v"]
        s = (g / np.sqrt(v + EPS)).astype(np.float32)
        t = (b - m * s).astype(np.float32)
        d = dict(t=t, pw=kw[f"e{i}_pw"].astype(np.float32),
                 pb=kw[f"e{i}_pb"].astype(np.float32))
        if i < 2:
            d["k"] = (kw[f"e{i}_k"].reshape(C, 9) * s[:, None]).astype(
                np.float32)
        else:
            d["M"] = (kw["e2_k"] * s[:, None]).astype(np.float32)
        out[i] = d
    return out


def _make_inmaps(xq, idx, elem, slot_modes, fold):
    in_maps = []
    ar = np.arange(128)
    ident = np.zeros((128, 128), np.float16)
    ident[ar, ar] = 1.0
    for c in range(NCORES):
        im = {"ident": ident}
        xs = np.zeros((NSLOT, C, PADHW), np.float16)
        for s in range(NSLOT):
            e = elem[c][s]
            if e >= 0:
                xs[s] = xq[e]
        im["xin"] = xs
        for s, (tm, has_m) in enumerate(slot_modes):
            if tm is None and not has_m:
                continue
            offs = _offsets(tm, has_m)
            ntap = len(offs)
            e = elem[c][s]
            t_e = idx[e] if e >= 0 else -1
            f = fold[t_e] if t_e >= 0 else None
            if ntap:
                dkd = np.zeros((128, NBLK, ntap, 128), np.float16)
                dkv = np.zeros((128, NBLK, ntap), np.float32)
            pmv = np.zeros((128, 2, 2, 128), np.float16)
            pwv = np.zeros((128, 2, 2, 128), np.float16)
            pfv = np.zeros((128, 4), np.float32)
            if f is not None:
                if t_e < 2:
                    k = f["k"]
                    de = t_e + 1
                    for t, (dy, dx, dil) in enumerate(offs):
                        q = (dy + 1) * 3 + (dx + 1)
                        if dil == de or (dy, dx) == (0, 0):
                            for b_ in range(NBLK):
                                col = k[b_ * 128 + ar, q]
                                dkd[ar, b_, t, ar] = col.astype(np.float16)
                                dkv[ar, b_, t] = col
                    if has_m:
                        for cb in range(2):
                            pmv[ar, cb, cb, ar] = \
                                k[cb * 128 + ar, 4].astype(np.float16)
                else:
                    M = f["M"]
                    for cb in range(2):
                        for ib in range(2):
                            pmv[:, cb, ib, :] = \
                                M[cb * 128:(cb + 1) * 128,
                                  ib * 128:(ib + 1) * 128].T.astype(
                                      np.float16)
                pfv[:, 0:2] = f["t"].reshape(2, 128).T
                P = f["pw"]
                for cb in range(2):
                    for ib in range(2):
                        pwv[:, cb, ib, :] = P[cb * 128:(cb + 1) * 128,
                                              ib * 128:(ib + 1) * 128].T \
                            .astype(np.float16)
                pfv[:, 2:4] = f["pb"].reshape(2, 128).T
            if ntap:
                im[f"dkd_{s}"] = dkd.reshape(128, -1)
                im[f"dkv_{s}"] = dkv.reshape(128, -1)
            if has_m:
                im[f"pm_{s}"] = pmv.reshape(128, -1)
            im[f"pw_{s}"] = pwv.reshape(128, -1)
            im[f"pf_{s}"] = pfv
        in_maps.append(im)
    return in_maps


def _pad_quant(x):
    xq = np.zeros((B, C, RH, R), np.float16)
    xq[:, :, PAD:PAD + H, PAD:PAD + W] = x.astype(np.float16)
    return xq.reshape(B, C, PADHW)


def kernel(**inputs):
    global LAST_NC
    inputs = {k: np.asarray(v) for k, v in inputs.items()}
    x = np.ascontiguousarray(inputs["x"], np.float32)
    rw = np.asarray(inputs["rw"], np.float32)
    rb = np.asarray(inputs["rb"], np.float32)
    pooled = x.mean(axis=(2, 3), dtype=np.float32)
    logits = pooled @ rw.T + rb
    idx = logits.argmax(-1)

    slot_modes, elem = _plan(idx)
    fold = _fold_params(inputs)
    xq = _pad_quant(x)
    in_maps = _make_inmaps(xq, idx, elem, slot_modes, fold)

    key = (slot_modes, tuple(sorted(KNOBS.items())))
    if key not in _CACHE:
        _CACHE[key] = build(slot_modes)
    nc = _CACHE[key]
    LAST_NC = nc
    res = run_bass_kernel_spmd(nc, in_maps, core_ids=list(range(NCORES)),
                               trace=False)
    out = np.zeros((B, C, H, W), np.float32)
    for c in range(NCORES):
        yo = res.results[c]["yout"]
        for s in range(NSLOT):
            e = elem[c][s]
            if e >= 0:
                out[e] = yo[s].astype(np.float32).reshape(C, H, W)
    return out

